# revision 19
# baseline (speedup 1.0000x reference)
"""Distributed Trainium2 Bass kernel for GQA attention (nn_Attention_27814208209106).

Sharding: 8 cores = 2 batches x 4 KV-head groups (7 q-heads + 1 kv head each).
v3: 4x256-token block pipeline with batched DMAs (host pre-packs every
weight/activation into partition-major monoliths so each logical transfer is
ONE descriptor: ~25 DMA issues total vs ~260 in v2 -- the SP sequencer at
~565ns/issue was gating the input stream). Small AllGathers (1.83MB out, one
per block) overlap compute; reciprocal broadcast + causal diag mask run on
the PE; o-proj in emb-partitioned layout.

All matmuls bf16 with f32 PSUM accumulation. PSUM banks are packed with
region-disjoint accumulation chains (8-bank budget).
"""

import numpy as np

import concourse.bass as bass
import concourse.mybir as mybir
import concourse.tile as tile
from concourse import bacc
from concourse.bass_utils import run_bass_kernel_spmd

P = 128
BW = 256           # token block width
NB = 4             # number of token blocks
THETA = 1000000.0
NEG = -30000.0

F32 = mybir.dt.float32
BF16 = mybir.dt.bfloat16


class Cfg:
    def __init__(self, T=1024, EMB=3584, NH=28, KVH=4, HD=128):
        self.T, self.EMB, self.NH, self.KVH, self.HD = T, EMB, NH, KVH, HD
        self.GQ = NH // KVH          # q heads per kv head (7)
        self.HG = self.GQ * HD       # per-core q width (896)
        self.NHD = NH * HD           # full qkv width (3584)
        self.EO = EMB // 4           # o-proj output slice per core (896)
        self.KT = EMB // P           # contraction tiles (28)
        self.ET = self.EO // P       # o-proj emb tiles (7)
        self.scale = HD ** -0.5


def build_kernel(cfg: Cfg):
    nc = bacc.Bacc(
        "TRN2",
        target_bir_lowering=False,
        debug=False,
        enable_asserts=False,
        num_devices=8,
    )

    KT, HG, EO = cfg.KT, cfg.HG, cfg.EO
    xb2 = nc.dram_tensor("xb2", [NB * P, KT * BW], BF16, kind="ExternalInput").ap()
    wq2 = nc.dram_tensor("wq2", [P, KT * HG], BF16, kind="ExternalInput").ap()
    wkv2 = nc.dram_tensor("wkv2", [P, 2 * KT * cfg.HD], BF16, kind="ExternalInput").ap()
    wo2 = nc.dram_tensor("wo2", [P, KT * EO], BF16, kind="ExternalInput").ap()
    cosT = nc.dram_tensor("cosT", [cfg.HD // 2, cfg.T], BF16, kind="ExternalInput").ap()
    sinT = nc.dram_tensor("sinT", [cfg.HD // 2, cfg.T], BF16, kind="ExternalInput").ap()
    oT_s = nc.dram_tensor("oT_s", [EO, cfg.T], F32, kind="ExternalOutput").ap()

    with tile.TileContext(nc) as tc:
        _body(tc, cfg, xb2, wq2, wkv2, wo2, cosT, sinT, oT_s)

    nc.compile()
    return nc


def _body(tc, cfg, xb2, wq2, wkv2, wo2, cosT, sinT, oT_s):
    nc = tc.nc
    H2 = cfg.HD // 2
    KT, HG, EO, NH, GQ = cfg.KT, cfg.HG, cfg.EO, cfg.NH, cfg.GQ

    from contextlib import ExitStack
    with ExitStack() as _st:
        ec = _st.enter_context
        constp = ec(tc.tile_pool(name="const", bufs=1))
        csp = ec(tc.tile_pool(name="cs", bufs=1))
        xTp = ec(tc.tile_pool(name="xT", bufs=2))
        wqp = ec(tc.tile_pool(name="wq", bufs=1))
        wkvp = ec(tc.tile_pool(name="wkv", bufs=1))
        wop = ec(tc.tile_pool(name="wo", bufs=1))
        kTp = ec(tc.tile_pool(name="kT", bufs=1))
        vp = ec(tc.tile_pool(name="vv", bufs=2 * NB))
        qTp = ec(tc.tile_pool(name="qT", bufs=2 * cfg.GQ))
        ptp = ec(tc.tile_pool(name="pt", bufs=8))
        recp = ec(tc.tile_pool(name="rec", bufs=2))
        qkvbp = ec(tc.tile_pool(name="qkvb", bufs=1))
        qkhp = ec(tc.tile_pool(name="qkh", bufs=2))
        osbp = ec(tc.tile_pool(name="osb", bufs=1))
        rtp = ec(tc.tile_pool(name="rtmp", bufs=2))
        dramp = ec(tc.tile_pool(name="dram", bufs=1, space="DRAM"))
        # PSUM: 8 banks x 2KB/partition, region-packed accumulation chains.
        pkvp = ec(tc.tile_pool(name="pkv", bufs=1, space="PSUM"))  # 1 bank
        pqp = ec(tc.tile_pool(name="pq", bufs=1, space="PSUM"))    # 1 bank
        plp = ec(tc.tile_pool(name="pl", bufs=2, space="PSUM"))    # 2 banks
        sprp = ec(tc.tile_pool(name="spr", bufs=1, space="PSUM"))  # 1 bank
        pavp = ec(tc.tile_pool(name="pav", bufs=1, space="PSUM"))  # 1 bank
        pop = ec(tc.tile_pool(name="po", bufs=2, space="PSUM"))    # 2 banks

        # ---- constants ----
        ident = constp.tile([P, P], BF16, name="ident")
        nc.gpsimd.memset(ident, 0.0)
        nc.gpsimd.affine_select(
            out=ident, in_=ident, compare_op=mybir.AluOpType.not_equal,
            fill=1.0, base=0, pattern=[[-1, P]], channel_multiplier=1,
        )
        # negd = diag(NEG)
        negd = constp.tile([P, P], BF16, name="negd")
        nc.gpsimd.memset(negd, 0.0)
        nc.gpsimd.affine_select(
            out=negd, in_=negd, compare_op=mybir.AluOpType.not_equal,
            fill=NEG, base=0, pattern=[[-1, P]], channel_multiplier=1,
        )
        # rlow[s, q] = 1 iff s > q  (strict lower triangle)
        rlow = constp.tile([P, P], BF16, name="rlow")
        nc.gpsimd.memset(rlow, 0.0)
        nc.gpsimd.affine_select(
            out=rlow, in_=rlow, compare_op=mybir.AluOpType.is_ge,
            fill=1.0, base=0, pattern=[[1, P]], channel_multiplier=-1,
        )
        ones_bf = constp.tile([P, 1], BF16, name="ones_bf")
        nc.vector.memset(ones_bf, 1.0)
        ones1 = constp.tile([1, P], BF16, name="ones1")
        nc.vector.memset(ones1, 1.0)
        wrm = constp.tile([P, BW], BF16, name="wrm")
        nc.vector.memset(wrm, 0.0)

        def warm_burst(n):
            pw = pop.tile([P, BW], F32, name="pwb", tag="po")
            for _ in range(n):
                nc.tensor.matmul(out=pw, lhsT=ident, rhs=wrm, start=True, stop=True)

        # PE warmup while first DMAs stream in
        warm_burst(40)

        # ---- batched DMA staging ----
        cos_sb = csp.tile([H2, cfg.T], BF16, name="cos_sb")
        sin_sb = csp.tile([H2, cfg.T], BF16, name="sin_sb")
        wkv_sb = wkvp.tile([P, 2 * KT * cfg.HD], BF16, name="wkv_sb")
        nc.sync.dma_start(wkv_sb, wkv2)
        nc.sync.dma_start(cos_sb, cosT)
        nc.sync.dma_start(sin_sb, sinT)

        xsb = []
        xsb.append(xTp.tile([P, KT * BW], BF16, name="xsb0", tag="xT"))
        nc.sync.dma_start(xsb[0], xb2[0:P, :])

        # wq in 4 chunks of 7 ke-tiles for q-proj(b0) streaming
        wq_sb = wqp.tile([P, KT * HG], BF16, name="wq_sb")
        WQC = KT // 4 * HG  # 6272
        for c in range(4):
            nc.sync.dma_start(wq_sb[:, c * WQC:(c + 1) * WQC],
                              wq2[:, c * WQC:(c + 1) * WQC])

        xsb.append(xTp.tile([P, KT * BW], BF16, name="xsb1", tag="xT"))
        nc.sync.dma_start(xsb[1], xb2[P:2 * P, :])

        wo_sb = wop.tile([P, KT * EO], BF16, name="wo_sb")
        WOC = KT // 2 * EO
        for c in range(2):
            nc.sync.dma_start(wo_sb[:, c * WOC:(c + 1) * WOC],
                              wo2[:, c * WOC:(c + 1) * WOC])

        for b in range(2, NB):
            t = xTp.tile([P, KT * BW], BF16, name=f"xsb{b}", tag="xT")
            nc.sync.dma_start(t, xb2[b * P:(b + 1) * P, :])
            xsb.append(t)

        def xt(ke, b):
            return xsb[b][:, ke * BW:(ke + 1) * BW]

        def wkh(ke):
            return wkv_sb[:, ke * cfg.HD:(ke + 1) * cfg.HD]

        def wvh(ke):
            return wkv_sb[:, KT * cfg.HD + ke * cfg.HD:KT * cfg.HD + (ke + 1) * cfg.HD]

        def wqh(ke, h):
            return wq_sb[:, ke * HG + h * P:ke * HG + (h + 1) * P]

        def woh(kt, e):
            return wo_sb[:, kt * EO + e * P:kt * EO + (e + 1) * P]

        kT = kTp.tile([P, cfg.T], BF16, name="kT")
        vts = [vp.tile([P, cfg.HD], BF16, name=f"v{i}", tag="v")
               for i in range(2 * NB)]

        cc_in = [dramp.tile([P, GQ * BW], BF16, name=f"cc_in{b}")
                 for b in range(NB)]
        cc_out = [dramp.tile([4 * P, GQ * BW], BF16, name=f"cc_out{b}")
                  for b in range(NB)]

        def rope_drain(psum, dst, t0, w, ang0=None):
            """dst[:, t0:t0+w] = rope(psum); psum [128, w] f32, dst bf16.
            ang0: global token offset for the cos/sin tables (default t0)."""
            ang0 = t0 if ang0 is None else ang0
            c = cos_sb[:, ang0:ang0 + w]
            s = sin_sb[:, ang0:ang0 + w]
            p1 = psum[0:H2, :]
            p2 = psum[H2:P, :]
            t1 = rtp.tile([H2, BW], F32, name="t1", tag="rt1")[:, :w]
            t2 = rtp.tile([H2, BW], F32, name="t2", tag="rt2")[:, :w]
            nc.vector.tensor_mul(t1, p1, c)
            nc.vector.tensor_mul(t2, p2, s)
            nc.vector.tensor_sub(dst[0:H2, t0:t0 + w], t1, t2)
            nc.vector.tensor_mul(t1, p2, c)
            nc.vector.tensor_mul(t2, p1, s)
            nc.vector.tensor_add(dst[H2:P, t0:t0 + w], t1, t2)

        qT = {}  # (h, b) -> tile

        def kv_proj(b):
            t0 = b * BW
            # one bank: k in cols [0:256), v sub-tiles in [256:384) / [384:512)
            bank = pkvp.tile([P, 512], F32, name="pkv", tag="pkv")
            psk = bank[:, 0:BW]
            for ke in range(KT):
                nc.tensor.matmul(out=psk, lhsT=wkh(ke), rhs=xt(ke, b),
                                 start=(ke == 0), stop=(ke == KT - 1))
            rope_drain(psk, kT, t0, BW)
            for sub in range(2):
                pv = bank[:, BW + sub * P:BW + (sub + 1) * P]
                for ke in range(KT):
                    nc.tensor.matmul(
                        out=pv, lhsT=xt(ke, b)[:, sub * P:(sub + 1) * P],
                        rhs=wvh(ke),
                        start=(ke == 0), stop=(ke == KT - 1))
                nc.scalar.copy(vts[2 * b + sub], pv)

        def q_proj_heads(b, heads, ke_outer):
            if ke_outer:
                # one bank per head (<=2 heads) so the two interleaved
                # accumulation chains never share a PSUM bank
                assert len(heads) <= 2
                slot = {h: pqp.tile([P, 512], F32, name=f"pqo{h}",
                                    tag="pq")[:, 0:BW] for h in heads}
                for ke in range(KT):
                    for h in heads:
                        nc.tensor.matmul(
                            out=slot[h], lhsT=wqh(ke, h), rhs=xt(ke, b),
                            start=(ke == 0), stop=(ke == KT - 1))
                for h in heads:
                    q = qTp.tile([P, BW], BF16, name=f"qT{h}_{b}", tag="qT")
                    rope_drain(slot[h], q, 0, BW, ang0=b * BW)
                    qT[(h, b)] = q
            else:
                # sequential heads: pack pairs into one bank (chains are
                # temporally disjoint; rope of h overlaps proj of h+1)
                bank = None
                for i, h in enumerate(heads):
                    if i % 2 == 0:
                        bank = pqp.tile([P, 512], F32, name=f"pq{h}", tag="pq")
                    ps = bank[:, (i % 2) * BW:(i % 2 + 1) * BW]
                    for ke in range(KT):
                        nc.tensor.matmul(
                            out=ps, lhsT=wqh(ke, h), rhs=xt(ke, b),
                            start=(ke == 0), stop=(ke == KT - 1))
                    q = qTp.tile([P, BW], BF16, name=f"qT{h}_{b}", tag="qT")
                    rope_drain(ps, q, 0, BW, ang0=b * BW)
                    qT[(h, b)] = q

        def attn_head(b, h, qkv_blk):
            """Attention for head h, token block b -> qkv_blk[:, h*BW:...]."""
            nsi = 2 * (b + 1)
            plbank = plp.tile([P, 512], F32, name="plb", tag="pl")
            pts = []
            for si in range(nsi):
                c0 = 0 if si <= 2 * b else P
                cw = BW - c0
                pl = plbank[:, (si % 2) * BW:(si % 2) * BW + cw]
                diag = si >= 2 * b
                nc.tensor.matmul(
                    out=pl,
                    lhsT=kT[:, si * P:(si + 1) * P],
                    rhs=qT[(h, b)][:, c0:BW],
                    start=True, stop=not diag,
                )
                if diag:
                    # add NEG to masked (s_local > q_local) entries of the
                    # first 128 columns of this si's valid q range
                    nc.tensor.matmul(
                        out=pl[:, 0:P], lhsT=negd, rhs=rlow,
                        start=False, stop=True,
                    )
                pt = ptp.tile([P, BW], BF16, name="pt", tag="pt")[:, :cw]
                nc.scalar.activation(
                    pt, pl, mybir.ActivationFunctionType.Exp, scale=cfg.scale,
                )
                pts.append((pt, c0, cw))

            # shared bank: recb at [0:256), denominators sp at [0:1, 256:512)
            sprbank = sprp.tile([P, 512], F32, name="spr", tag="spr")
            sp = sprbank[0:1, BW:2 * BW]
            for si, (pt, c0, cw) in enumerate(pts):
                nc.tensor.matmul(
                    out=sp[:, c0:c0 + cw], lhsT=ones_bf, rhs=pt,
                    start=(si == 0), stop=(si == nsi - 1),
                )
            rec_bf = recp.tile([1, BW], BF16, name="rec", tag="rec")
            with nc.allow_low_precision("softmax reciprocal feeds bf16 matmul"):
                nc.vector.reciprocal(out=rec_bf, in_=sp)
            recb_ps = sprbank[:, 0:BW]
            nc.tensor.matmul(out=recb_ps, lhsT=ones1, rhs=rec_bf,
                             start=True, stop=True)
            # TensorTensor may read only one PSUM operand: drain recb to SBUF
            recb = recp.tile([P, BW], BF16, name="recb", tag="recb")
            nc.vector.tensor_copy(recb, recb_ps)

            pav = pavp.tile([P, BW], F32, name="pav", tag="pav")
            for si, (pt, c0, cw) in enumerate(pts):
                nc.tensor.matmul(
                    out=pav[:, c0:c0 + cw], lhsT=vts[si], rhs=pt,
                    start=(si == 0), stop=(si == nsi - 1),
                )
            nc.vector.tensor_mul(qkv_blk[:, h * BW:(h + 1) * BW], pav, recb)

        def allgather(b):
            nc.gpsimd.collective_compute(
                "AllGather",
                mybir.AluOpType.bypass,
                replica_groups=[[0, 1, 2, 3], [4, 5, 6, 7]],
                ins=[cc_in[b].opt()],
                outs=[cc_out[b].opt()],
            )

        def oproj(b, qkh):
            osb = osbp.tile([P, cfg.ET * BW], F32, name=f"osb{b}", tag="osb")
            for e in range(cfg.ET):
                po = pop.tile([P, BW], F32, name="po", tag="po")
                for kt in range(NH):
                    nc.tensor.matmul(
                        out=po, lhsT=woh(kt, e), rhs=qkh[:, kt * BW:(kt + 1) * BW],
                        start=(kt == 0), stop=(kt == NH - 1),
                    )
                nc.scalar.copy(osb[:, e * BW:(e + 1) * BW], po)
            nc.sync.dma_start(
                oT_s[:, b * BW:(b + 1) * BW].rearrange("(e p) c -> p e c", p=P),
                osb)

        def attn_block(b, heads):
            qkv_blk = qkvbp.tile([P, GQ * BW], BF16, name=f"qkvb{b}", tag="qkvb")
            for h in heads:
                attn_head(b, h, qkv_blk)
            nc.sync.dma_start(cc_in[b], qkv_blk)
            allgather(b)
            # prefetch gathered qkv^T: one strided DMA [4*128, 7*256] -> [128, 28*256]
            qkh = qkhp.tile([P, NH * BW], BF16, name=f"qkh{b}", tag="qkh")
            nc.sync.dma_start(
                qkh, cc_out[b][:, :].rearrange("(r p) f -> p r f", p=P))
            return qkh

        # ================= pipeline =================
        qkhs = []
        for b in range(NB):
            kv_proj(b)
            q_proj_heads(b, list(range(GQ)), ke_outer=False)
            qkhs.append(attn_block(b, list(range(GQ))))
        for b in range(NB):
            oproj(b, qkhs[b])


# ======================= host side =======================

_NC_CACHE = {}


def _get_nc(cfg_key=None):
    if cfg_key not in _NC_CACHE:
        _NC_CACHE[cfg_key] = build_kernel(Cfg())
    return _NC_CACHE[cfg_key]


def _rope_tables(segment_ids, cur_ind, T, HD):
    valid = (np.asarray(segment_ids) != 0)
    pos = np.cumsum(valid, axis=-1) - 1 + int(cur_ind)  # [B, T]
    frac = 2.0 * np.arange(HD // 2, dtype=np.float64) / HD
    timescale = THETA ** frac
    ang = pos[..., None].astype(np.float64) / timescale  # [B, T, HD/2]
    cosT = np.transpose(np.cos(ang), (0, 2, 1)).astype(np.float32)  # [B, HD/2, T]
    sinT = np.transpose(np.sin(ang), (0, 2, 1)).astype(np.float32)
    return cosT, sinT


def _pack_km(w, P=128):
    """[K, M] -> [P, (K/P)*M]: tile ke on rows -> partition-major columns."""
    K, M = w.shape
    kt = K // P
    return np.ascontiguousarray(
        w.reshape(kt, P, M).transpose(1, 0, 2).reshape(P, kt * M))


def prepare_in_maps(inputs, cfg=None):
    import ml_dtypes
    bf16 = ml_dtypes.bfloat16
    cfg = cfg or Cfg()
    x = np.asarray(inputs["x"], dtype=np.float32)
    wq = np.asarray(inputs["wq"], dtype=np.float32).astype(bf16)
    wk = np.asarray(inputs["wk"], dtype=np.float32).astype(bf16)
    wv = np.asarray(inputs["wv"], dtype=np.float32).astype(bf16)
    wo = np.asarray(inputs["wo"], dtype=np.float32).astype(bf16)
    seg = np.asarray(inputs["segment_ids"])
    cur = int(np.asarray(inputs["cur_ind"]))

    B, T, EMB = x.shape
    assert (B, T, EMB) == (2, cfg.T, cfg.EMB)
    HG, HD, EO, KT = cfg.HG, cfg.HD, cfg.EO, cfg.KT
    cosT, sinT = _rope_tables(seg, cur, T, HD)
    xT = np.transpose(x, (0, 2, 1)).astype(bf16)  # [B, EMB, T]
    # xb2: [B, NB*P, KT*BW]; block b rows [b*P:(b+1)*P], x tile (ke,b) at
    # cols [ke*BW:(ke+1)*BW]
    xb2 = np.ascontiguousarray(
        xT.reshape(B, KT, P, NB, BW).transpose(0, 3, 2, 1, 4).reshape(
            B, NB * P, KT * BW))

    in_maps = []
    for c in range(8):
        b, j = c // 4, c % 4
        wkj = np.ascontiguousarray(wk[:, j * HD:(j + 1) * HD])
        wvj = np.ascontiguousarray(wv[:, j * HD:(j + 1) * HD])
        wkv2 = np.concatenate([_pack_km(wkj), _pack_km(wvj)], axis=1)
        in_maps.append({
            "xb2": xb2[b],
            "wq2": _pack_km(np.ascontiguousarray(wq[:, j * HG:(j + 1) * HG])),
            "wkv2": np.ascontiguousarray(wkv2),
            "wo2": _pack_km(np.ascontiguousarray(wo[:, j * EO:(j + 1) * EO])),
            "cosT": np.ascontiguousarray(cosT[b]).astype(bf16),
            "sinT": np.ascontiguousarray(sinT[b]).astype(bf16),
        })
    return in_maps


def assemble_out(results, cfg=None):
    cfg = cfg or Cfg()
    out = np.empty((2, cfg.T, cfg.EMB), np.float32)
    for c in range(8):
        b, j = c // 4, c % 4
        out[b, :, j * cfg.EO:(j + 1) * cfg.EO] = results[c]["oT_s"].T
    return out


def kernel(**inputs):
    cfg = Cfg()
    in_maps = prepare_in_maps(inputs, cfg)
    nc = _get_nc()
    res = run_bass_kernel_spmd(nc, in_maps, core_ids=list(range(8)))
    return assemble_out(res.results, cfg)


# revision 21
# speedup vs baseline: 1.0248x; 1.0248x over previous
"""Distributed Trainium2 Bass kernel for GQA attention (nn_Attention_27814208209106).

Sharding: 8 cores = 2 batches x 4 KV-head groups (7 q-heads + 1 kv head each).
v3: 4x256-token block pipeline with batched DMAs (host pre-packs every
weight/activation into partition-major monoliths so each logical transfer is
ONE descriptor: ~25 DMA issues total vs ~260 in v2 -- the SP sequencer at
~565ns/issue was gating the input stream). Small AllGathers (1.83MB out, one
per block) overlap compute; reciprocal broadcast + causal diag mask run on
the PE; o-proj in emb-partitioned layout.

All matmuls bf16 with f32 PSUM accumulation. PSUM banks are packed with
region-disjoint accumulation chains (8-bank budget).
"""

import numpy as np

import concourse.bass as bass
import concourse.mybir as mybir
import concourse.tile as tile
from concourse import bacc
from concourse.bass_utils import run_bass_kernel_spmd

P = 128
BW = 256           # token block width
NB = 4             # number of token blocks
THETA = 1000000.0
NEG = -30000.0

F32 = mybir.dt.float32
BF16 = mybir.dt.bfloat16


class Cfg:
    def __init__(self, T=1024, EMB=3584, NH=28, KVH=4, HD=128):
        self.T, self.EMB, self.NH, self.KVH, self.HD = T, EMB, NH, KVH, HD
        self.GQ = NH // KVH          # q heads per kv head (7)
        self.HG = self.GQ * HD       # per-core q width (896)
        self.NHD = NH * HD           # full qkv width (3584)
        self.EO = EMB // 4           # o-proj output slice per core (896)
        self.KT = EMB // P           # contraction tiles (28)
        self.ET = self.EO // P       # o-proj emb tiles (7)
        self.scale = HD ** -0.5


def build_kernel(cfg: Cfg):
    nc = bacc.Bacc(
        "TRN2",
        target_bir_lowering=False,
        debug=False,
        enable_asserts=False,
        num_devices=8,
    )

    KT, HG, EO = cfg.KT, cfg.HG, cfg.EO
    xb2 = nc.dram_tensor("xb2", [NB * P, KT * BW], BF16, kind="ExternalInput").ap()
    wq2 = nc.dram_tensor("wq2", [P, KT * HG], BF16, kind="ExternalInput").ap()
    wkv2 = nc.dram_tensor("wkv2", [P, 2 * KT * cfg.HD], BF16, kind="ExternalInput").ap()
    wo2 = nc.dram_tensor("wo2", [P, KT * EO], BF16, kind="ExternalInput").ap()
    cosT = nc.dram_tensor("cosT", [cfg.HD // 2, cfg.T], BF16, kind="ExternalInput").ap()
    sinT = nc.dram_tensor("sinT", [cfg.HD // 2, cfg.T], BF16, kind="ExternalInput").ap()
    oT_s = nc.dram_tensor("oT_s", [EO, cfg.T], BF16, kind="ExternalOutput").ap()

    with tile.TileContext(nc) as tc:
        _body(tc, cfg, xb2, wq2, wkv2, wo2, cosT, sinT, oT_s)

    nc.compile()
    return nc


def _body(tc, cfg, xb2, wq2, wkv2, wo2, cosT, sinT, oT_s):
    nc = tc.nc
    H2 = cfg.HD // 2
    KT, HG, EO, NH, GQ = cfg.KT, cfg.HG, cfg.EO, cfg.NH, cfg.GQ

    from contextlib import ExitStack
    with ExitStack() as _st:
        ec = _st.enter_context
        constp = ec(tc.tile_pool(name="const", bufs=1))
        csp = ec(tc.tile_pool(name="cs", bufs=1))
        xTp = ec(tc.tile_pool(name="xT", bufs=2))
        wqp = ec(tc.tile_pool(name="wq", bufs=1))
        wkvp = ec(tc.tile_pool(name="wkv", bufs=1))
        wop = ec(tc.tile_pool(name="wo", bufs=1))
        kTp = ec(tc.tile_pool(name="kT", bufs=1))
        vp = ec(tc.tile_pool(name="vv", bufs=2 * NB))
        qTp = ec(tc.tile_pool(name="qT", bufs=2 * cfg.GQ))
        ptp = ec(tc.tile_pool(name="pt", bufs=8))
        recp = ec(tc.tile_pool(name="rec", bufs=2))
        qkvbp = ec(tc.tile_pool(name="qkvb", bufs=2))
        qkhp = ec(tc.tile_pool(name="qkh", bufs=2))
        osbp = ec(tc.tile_pool(name="osb", bufs=1))
        rtp = ec(tc.tile_pool(name="rtmp", bufs=2))
        dramp = ec(tc.tile_pool(name="dram", bufs=1, space="DRAM"))
        # PSUM: 8 banks x 2KB/partition, region-packed accumulation chains.
        pkvp = ec(tc.tile_pool(name="pkv", bufs=1, space="PSUM"))  # 1 bank
        pqp = ec(tc.tile_pool(name="pq", bufs=1, space="PSUM"))    # 1 bank
        plp = ec(tc.tile_pool(name="pl", bufs=2, space="PSUM"))    # 2 banks
        sprp = ec(tc.tile_pool(name="spr", bufs=1, space="PSUM"))  # 1 bank
        pavp = ec(tc.tile_pool(name="pav", bufs=1, space="PSUM"))  # 1 bank
        pop = ec(tc.tile_pool(name="po", bufs=2, space="PSUM"))    # 2 banks

        # ---- constants ----
        ident = constp.tile([P, P], BF16, name="ident")
        nc.gpsimd.memset(ident, 0.0)
        nc.gpsimd.affine_select(
            out=ident, in_=ident, compare_op=mybir.AluOpType.not_equal,
            fill=1.0, base=0, pattern=[[-1, P]], channel_multiplier=1,
        )
        # negd = diag(NEG)
        negd = constp.tile([P, P], BF16, name="negd")
        nc.gpsimd.memset(negd, 0.0)
        nc.gpsimd.affine_select(
            out=negd, in_=negd, compare_op=mybir.AluOpType.not_equal,
            fill=NEG, base=0, pattern=[[-1, P]], channel_multiplier=1,
        )
        # rlow[s, q] = 1 iff s > q  (strict lower triangle)
        rlow = constp.tile([P, P], BF16, name="rlow")
        nc.gpsimd.memset(rlow, 0.0)
        nc.gpsimd.affine_select(
            out=rlow, in_=rlow, compare_op=mybir.AluOpType.is_ge,
            fill=1.0, base=0, pattern=[[1, P]], channel_multiplier=-1,
        )
        ones_bf = constp.tile([P, 1], BF16, name="ones_bf")
        nc.vector.memset(ones_bf, 1.0)
        ones1 = constp.tile([1, P], BF16, name="ones1")
        nc.vector.memset(ones1, 1.0)
        wrm = constp.tile([P, BW], BF16, name="wrm")
        nc.vector.memset(wrm, 0.0)

        def warm_burst(n):
            pw = pop.tile([P, BW], F32, name="pwb", tag="po")
            for _ in range(n):
                nc.tensor.matmul(out=pw, lhsT=ident, rhs=wrm, start=True, stop=True)

        # PE warmup while first DMAs stream in
        warm_burst(40)

        # ---- batched DMA staging ----
        cos_sb = csp.tile([H2, cfg.T], BF16, name="cos_sb")
        sin_sb = csp.tile([H2, cfg.T], BF16, name="sin_sb")
        wkv_sb = wkvp.tile([P, 2 * KT * cfg.HD], BF16, name="wkv_sb")
        nc.sync.dma_start(wkv_sb, wkv2)
        nc.sync.dma_start(cos_sb, cosT)
        nc.sync.dma_start(sin_sb, sinT)

        xsb = []
        xsb.append(xTp.tile([P, KT * BW], BF16, name="xsb0", tag="xT"))
        nc.sync.dma_start(xsb[0], xb2[0:P, :])

        # wq in 4 chunks of 7 ke-tiles for q-proj(b0) streaming
        wq_sb = wqp.tile([P, KT * HG], BF16, name="wq_sb")
        WQC = KT // 4 * HG  # 6272
        for c in range(4):
            nc.sync.dma_start(wq_sb[:, c * WQC:(c + 1) * WQC],
                              wq2[:, c * WQC:(c + 1) * WQC])

        xsb.append(xTp.tile([P, KT * BW], BF16, name="xsb1", tag="xT"))
        nc.sync.dma_start(xsb[1], xb2[P:2 * P, :])

        wo_sb = wop.tile([P, KT * EO], BF16, name="wo_sb")
        WOC = KT // 2 * EO
        for c in range(2):
            nc.sync.dma_start(wo_sb[:, c * WOC:(c + 1) * WOC],
                              wo2[:, c * WOC:(c + 1) * WOC])

        for b in range(2, NB):
            t = xTp.tile([P, KT * BW], BF16, name=f"xsb{b}", tag="xT")
            nc.sync.dma_start(t, xb2[b * P:(b + 1) * P, :])
            xsb.append(t)

        def xt(ke, b):
            return xsb[b][:, ke * BW:(ke + 1) * BW]

        def wkh(ke):
            return wkv_sb[:, ke * cfg.HD:(ke + 1) * cfg.HD]

        def wvh(ke):
            return wkv_sb[:, KT * cfg.HD + ke * cfg.HD:KT * cfg.HD + (ke + 1) * cfg.HD]

        def wqh(ke, h):
            return wq_sb[:, ke * HG + h * P:ke * HG + (h + 1) * P]

        def woh(kt, e):
            return wo_sb[:, kt * EO + e * P:kt * EO + (e + 1) * P]

        kT = kTp.tile([P, cfg.T], BF16, name="kT")
        vts = [vp.tile([P, cfg.HD], BF16, name=f"v{i}", tag="v")
               for i in range(2 * NB)]

        cc_in = [dramp.tile([P, GQ * BW], BF16, name=f"cc_in{b}")
                 for b in range(NB)]
        cc_out = [dramp.tile([4 * P, GQ * BW], BF16, name=f"cc_out{b}")
                  for b in range(NB)]

        def rope_drain(psum, dst, t0, w, ang0=None):
            """dst[:, t0:t0+w] = rope(psum); psum [128, w] f32, dst bf16.
            ang0: global token offset for the cos/sin tables (default t0)."""
            ang0 = t0 if ang0 is None else ang0
            c = cos_sb[:, ang0:ang0 + w]
            s = sin_sb[:, ang0:ang0 + w]
            p1 = psum[0:H2, :]
            p2 = psum[H2:P, :]
            t1 = rtp.tile([H2, BW], F32, name="t1", tag="rt1")[:, :w]
            t2 = rtp.tile([H2, BW], F32, name="t2", tag="rt2")[:, :w]
            nc.vector.tensor_mul(t1, p1, c)
            nc.vector.tensor_mul(t2, p2, s)
            nc.vector.tensor_sub(dst[0:H2, t0:t0 + w], t1, t2)
            nc.vector.tensor_mul(t1, p2, c)
            nc.vector.tensor_mul(t2, p1, s)
            nc.vector.tensor_add(dst[H2:P, t0:t0 + w], t1, t2)

        qT = {}  # (h, b) -> tile

        def kv_proj(b):
            t0 = b * BW
            # one bank: k in cols [0:256), v sub-tiles in [256:384) / [384:512)
            bank = pkvp.tile([P, 512], F32, name="pkv", tag="pkv")
            psk = bank[:, 0:BW]
            for ke in range(KT):
                nc.tensor.matmul(out=psk, lhsT=wkh(ke), rhs=xt(ke, b),
                                 start=(ke == 0), stop=(ke == KT - 1))
            rope_drain(psk, kT, t0, BW)
            for sub in range(2):
                pv = bank[:, BW + sub * P:BW + (sub + 1) * P]
                for ke in range(KT):
                    nc.tensor.matmul(
                        out=pv, lhsT=xt(ke, b)[:, sub * P:(sub + 1) * P],
                        rhs=wvh(ke),
                        start=(ke == 0), stop=(ke == KT - 1))
                nc.scalar.copy(vts[2 * b + sub], pv)

        def q_proj_heads(b, heads, ke_outer):
            if ke_outer:
                # one bank per head (<=2 heads) so the two interleaved
                # accumulation chains never share a PSUM bank
                assert len(heads) <= 2
                slot = {h: pqp.tile([P, 512], F32, name=f"pqo{h}",
                                    tag="pq")[:, 0:BW] for h in heads}
                for ke in range(KT):
                    for h in heads:
                        nc.tensor.matmul(
                            out=slot[h], lhsT=wqh(ke, h), rhs=xt(ke, b),
                            start=(ke == 0), stop=(ke == KT - 1))
                for h in heads:
                    q = qTp.tile([P, BW], BF16, name=f"qT{h}_{b}", tag="qT")
                    rope_drain(slot[h], q, 0, BW, ang0=b * BW)
                    qT[(h, b)] = q
            else:
                # sequential heads: pack pairs into one bank (chains are
                # temporally disjoint; rope of h overlaps proj of h+1)
                bank = None
                for i, h in enumerate(heads):
                    if i % 2 == 0:
                        bank = pqp.tile([P, 512], F32, name=f"pq{h}", tag="pq")
                    ps = bank[:, (i % 2) * BW:(i % 2 + 1) * BW]
                    for ke in range(KT):
                        nc.tensor.matmul(
                            out=ps, lhsT=wqh(ke, h), rhs=xt(ke, b),
                            start=(ke == 0), stop=(ke == KT - 1))
                    q = qTp.tile([P, BW], BF16, name=f"qT{h}_{b}", tag="qT")
                    rope_drain(ps, q, 0, BW, ang0=b * BW)
                    qT[(h, b)] = q

        def attn_head(b, h, qkv_blk):
            """Attention for head h, token block b -> qkv_blk[:, h*BW:...]."""
            nsi = 2 * (b + 1)
            plbank = plp.tile([P, 512], F32, name="plb", tag="pl")
            pts = []
            for si in range(nsi):
                c0 = 0 if si <= 2 * b else P
                cw = BW - c0
                pl = plbank[:, (si % 2) * BW:(si % 2) * BW + cw]
                diag = si >= 2 * b
                nc.tensor.matmul(
                    out=pl,
                    lhsT=kT[:, si * P:(si + 1) * P],
                    rhs=qT[(h, b)][:, c0:BW],
                    start=True, stop=not diag,
                )
                if diag:
                    # add NEG to masked (s_local > q_local) entries of the
                    # first 128 columns of this si's valid q range
                    nc.tensor.matmul(
                        out=pl[:, 0:P], lhsT=negd, rhs=rlow,
                        start=False, stop=True,
                    )
                pt = ptp.tile([P, BW], BF16, name="pt", tag="pt")[:, :cw]
                nc.scalar.activation(
                    pt, pl, mybir.ActivationFunctionType.Exp, scale=cfg.scale,
                )
                pts.append((pt, c0, cw))

            # shared bank: recb at [0:256), denominators sp at [0:1, 256:512)
            sprbank = sprp.tile([P, 512], F32, name="spr", tag="spr")
            sp = sprbank[0:1, BW:2 * BW]
            for si, (pt, c0, cw) in enumerate(pts):
                nc.tensor.matmul(
                    out=sp[:, c0:c0 + cw], lhsT=ones_bf, rhs=pt,
                    start=(si == 0), stop=(si == nsi - 1),
                )
            rec_bf = recp.tile([1, BW], BF16, name="rec", tag="rec")
            with nc.allow_low_precision("softmax reciprocal feeds bf16 matmul"):
                nc.vector.reciprocal(out=rec_bf, in_=sp)
            recb_ps = sprbank[:, 0:BW]
            nc.tensor.matmul(out=recb_ps, lhsT=ones1, rhs=rec_bf,
                             start=True, stop=True)
            # TensorTensor may read only one PSUM operand: drain recb to SBUF
            recb = recp.tile([P, BW], BF16, name="recb", tag="recb")
            nc.vector.tensor_copy(recb, recb_ps)

            pav = pavp.tile([P, BW], F32, name="pav", tag="pav")
            for si, (pt, c0, cw) in enumerate(pts):
                nc.tensor.matmul(
                    out=pav[:, c0:c0 + cw], lhsT=vts[si], rhs=pt,
                    start=(si == 0), stop=(si == nsi - 1),
                )
            nc.vector.tensor_mul(qkv_blk[:, h * BW:(h + 1) * BW], pav, recb)

        def allgather(b):
            nc.gpsimd.collective_compute(
                "AllGather",
                mybir.AluOpType.bypass,
                replica_groups=[[0, 1, 2, 3], [4, 5, 6, 7]],
                ins=[cc_in[b].opt()],
                outs=[cc_out[b].opt()],
            )

        def fetch_qkh(b):
            # gathered qkv^T: one strided DMA [4*128, 7*256] -> [128, 28*256]
            # on the GpSimd queue (waits AG done without blocking Sync DMAs)
            qkh = qkhp.tile([P, NH * BW], BF16, name=f"qkh{b}", tag="qkh")
            nc.gpsimd.dma_start(
                qkh, cc_out[b][:, :].rearrange("(r p) f -> p r f", p=P))
            return qkh

        def oproj(b, qkh):
            osb = osbp.tile([P, cfg.ET * BW], BF16, name=f"osb{b}", tag="osb")
            for e in range(cfg.ET):
                po = pop.tile([P, BW], F32, name="po", tag="po")
                for kt in range(NH):
                    nc.tensor.matmul(
                        out=po, lhsT=woh(kt, e), rhs=qkh[:, kt * BW:(kt + 1) * BW],
                        start=(kt == 0), stop=(kt == NH - 1),
                    )
                nc.scalar.copy(osb[:, e * BW:(e + 1) * BW], po)
            nc.sync.dma_start(
                oT_s[:, b * BW:(b + 1) * BW].rearrange("(e p) c -> p e c", p=P),
                osb)

        def attn_block(b, heads):
            qkv_blk = qkvbp.tile([P, GQ * BW], BF16, name=f"qkvb{b}", tag="qkvb")
            for h in heads:
                attn_head(b, h, qkv_blk)
            nc.sync.dma_start(cc_in[b], qkv_blk)
            allgather(b)

        # ================= pipeline =================
        qkhs = {}
        for b in range(NB):
            kv_proj(b)
            q_proj_heads(b, list(range(GQ)), ke_outer=False)
            attn_block(b, list(range(GQ)))
            if b < 2:
                qkhs[b] = fetch_qkh(b)
        for b in range(NB):
            oproj(b, qkhs[b])
            if b + 2 in range(NB):
                qkhs[b + 2] = fetch_qkh(b + 2)


# ======================= host side =======================

_NC_CACHE = {}


def _get_nc(cfg_key=None):
    if cfg_key not in _NC_CACHE:
        _NC_CACHE[cfg_key] = build_kernel(Cfg())
    return _NC_CACHE[cfg_key]


def _rope_tables(segment_ids, cur_ind, T, HD):
    valid = (np.asarray(segment_ids) != 0)
    pos = np.cumsum(valid, axis=-1) - 1 + int(cur_ind)  # [B, T]
    frac = 2.0 * np.arange(HD // 2, dtype=np.float64) / HD
    timescale = THETA ** frac
    ang = pos[..., None].astype(np.float64) / timescale  # [B, T, HD/2]
    cosT = np.transpose(np.cos(ang), (0, 2, 1)).astype(np.float32)  # [B, HD/2, T]
    sinT = np.transpose(np.sin(ang), (0, 2, 1)).astype(np.float32)
    return cosT, sinT


def _pack_km(w, P=128):
    """[K, M] -> [P, (K/P)*M]: tile ke on rows -> partition-major columns."""
    K, M = w.shape
    kt = K // P
    return np.ascontiguousarray(
        w.reshape(kt, P, M).transpose(1, 0, 2).reshape(P, kt * M))


def prepare_in_maps(inputs, cfg=None):
    import ml_dtypes
    bf16 = ml_dtypes.bfloat16
    cfg = cfg or Cfg()
    x = np.asarray(inputs["x"], dtype=np.float32)
    wq = np.asarray(inputs["wq"], dtype=np.float32).astype(bf16)
    wk = np.asarray(inputs["wk"], dtype=np.float32).astype(bf16)
    wv = np.asarray(inputs["wv"], dtype=np.float32).astype(bf16)
    wo = np.asarray(inputs["wo"], dtype=np.float32).astype(bf16)
    seg = np.asarray(inputs["segment_ids"])
    cur = int(np.asarray(inputs["cur_ind"]))

    B, T, EMB = x.shape
    assert (B, T, EMB) == (2, cfg.T, cfg.EMB)
    HG, HD, EO, KT = cfg.HG, cfg.HD, cfg.EO, cfg.KT
    cosT, sinT = _rope_tables(seg, cur, T, HD)
    xT = np.transpose(x, (0, 2, 1)).astype(bf16)  # [B, EMB, T]
    # xb2: [B, NB*P, KT*BW]; block b rows [b*P:(b+1)*P], x tile (ke,b) at
    # cols [ke*BW:(ke+1)*BW]
    xb2 = np.ascontiguousarray(
        xT.reshape(B, KT, P, NB, BW).transpose(0, 3, 2, 1, 4).reshape(
            B, NB * P, KT * BW))

    in_maps = []
    for c in range(8):
        b, j = c // 4, c % 4
        wkj = np.ascontiguousarray(wk[:, j * HD:(j + 1) * HD])
        wvj = np.ascontiguousarray(wv[:, j * HD:(j + 1) * HD])
        wkv2 = np.concatenate([_pack_km(wkj), _pack_km(wvj)], axis=1)
        in_maps.append({
            "xb2": xb2[b],
            "wq2": _pack_km(np.ascontiguousarray(wq[:, j * HG:(j + 1) * HG])),
            "wkv2": np.ascontiguousarray(wkv2),
            "wo2": _pack_km(np.ascontiguousarray(wo[:, j * EO:(j + 1) * EO])),
            "cosT": np.ascontiguousarray(cosT[b]).astype(bf16),
            "sinT": np.ascontiguousarray(sinT[b]).astype(bf16),
        })
    return in_maps


def assemble_out(results, cfg=None):
    cfg = cfg or Cfg()
    out = np.empty((2, cfg.T, cfg.EMB), np.float32)
    for c in range(8):
        b, j = c // 4, c % 4
        out[b, :, j * cfg.EO:(j + 1) * cfg.EO] = results[c]["oT_s"].T.astype(np.float32)
    return out


def kernel(**inputs):
    cfg = Cfg()
    in_maps = prepare_in_maps(inputs, cfg)
    nc = _get_nc()
    res = run_bass_kernel_spmd(nc, in_maps, core_ids=list(range(8)))
    return assemble_out(res.results, cfg)


# revision 25
# speedup vs baseline: 1.0921x; 1.0656x over previous
"""Distributed Trainium2 Bass kernel for GQA attention (nn_Attention_27814208209106).

Sharding: 8 cores = 2 batches x 4 KV-head groups (7 q-heads + 1 kv head each).
v3: 4x256-token block pipeline with batched DMAs (host pre-packs every
weight/activation into partition-major monoliths so each logical transfer is
ONE descriptor: ~25 DMA issues total vs ~260 in v2 -- the SP sequencer at
~565ns/issue was gating the input stream). Small AllGathers (1.83MB out, one
per block) overlap compute; reciprocal broadcast + causal diag mask run on
the PE; o-proj in emb-partitioned layout.

All matmuls bf16 with f32 PSUM accumulation. PSUM banks are packed with
region-disjoint accumulation chains (8-bank budget).
"""

import numpy as np

import concourse.bass as bass
import concourse.mybir as mybir
import concourse.tile as tile
from concourse import bacc
from concourse.bass_utils import run_bass_kernel_spmd

P = 128
BW = 256           # token block width
NB = 4             # number of token blocks
THETA = 1000000.0
NEG = -30000.0

F32 = mybir.dt.float32
BF16 = mybir.dt.bfloat16


class Cfg:
    def __init__(self, T=1024, EMB=3584, NH=28, KVH=4, HD=128):
        self.T, self.EMB, self.NH, self.KVH, self.HD = T, EMB, NH, KVH, HD
        self.GQ = NH // KVH          # q heads per kv head (7)
        self.HG = self.GQ * HD       # per-core q width (896)
        self.NHD = NH * HD           # full qkv width (3584)
        self.EO = EMB // 4           # o-proj output slice per core (896)
        self.KT = EMB // P           # contraction tiles (28)
        self.ET = self.EO // P       # o-proj emb tiles (7)
        self.scale = HD ** -0.5


def build_kernel(cfg: Cfg):
    nc = bacc.Bacc(
        "TRN2",
        target_bir_lowering=False,
        debug=False,
        enable_asserts=False,
        num_devices=8,
    )

    KT, HG, EO = cfg.KT, cfg.HG, cfg.EO
    xb2 = nc.dram_tensor("xb2", [NB * P, KT * BW], BF16, kind="ExternalInput").ap()
    wq2 = nc.dram_tensor("wq2", [P, KT * HG], BF16, kind="ExternalInput").ap()
    wkv2 = nc.dram_tensor("wkv2", [P, 2 * KT * cfg.HD], BF16, kind="ExternalInput").ap()
    wo2 = nc.dram_tensor("wo2", [P, KT * EO], BF16, kind="ExternalInput").ap()
    cosT = nc.dram_tensor("cosT", [cfg.HD // 2, cfg.T], BF16, kind="ExternalInput").ap()
    sinT = nc.dram_tensor("sinT", [cfg.HD // 2, cfg.T], BF16, kind="ExternalInput").ap()
    oT_s = nc.dram_tensor("oT_s", [EO, cfg.T], BF16, kind="ExternalOutput").ap()

    with tile.TileContext(nc) as tc:
        _body(tc, cfg, xb2, wq2, wkv2, wo2, cosT, sinT, oT_s)

    nc.compile()
    return nc


def _body(tc, cfg, xb2, wq2, wkv2, wo2, cosT, sinT, oT_s):
    nc = tc.nc
    H2 = cfg.HD // 2
    KT, HG, EO, NH, GQ = cfg.KT, cfg.HG, cfg.EO, cfg.NH, cfg.GQ

    from contextlib import ExitStack
    with ExitStack() as _st:
        ec = _st.enter_context
        constp = ec(tc.tile_pool(name="const", bufs=1))
        csp = ec(tc.tile_pool(name="cs", bufs=1))
        xTp = ec(tc.tile_pool(name="xT", bufs=2))
        wqp = ec(tc.tile_pool(name="wq", bufs=1))
        wkvp = ec(tc.tile_pool(name="wkv", bufs=1))
        wop = ec(tc.tile_pool(name="wo", bufs=1))
        kTp = ec(tc.tile_pool(name="kT", bufs=1))
        vp = ec(tc.tile_pool(name="vv", bufs=2 * NB))
        qTp2 = ec(tc.tile_pool(name="qTpair", bufs=6))
        qTs2 = ec(tc.tile_pool(name="qTsingle", bufs=2))
        ptp = ec(tc.tile_pool(name="pt", bufs=8))
        recp = ec(tc.tile_pool(name="rec", bufs=1))
        qkvbp = ec(tc.tile_pool(name="qkvb", bufs=2))
        qkhp = ec(tc.tile_pool(name="qkh", bufs=2))
        osbp = ec(tc.tile_pool(name="osb", bufs=1))
        rtp = ec(tc.tile_pool(name="rtmp", bufs=1))
        dramp = ec(tc.tile_pool(name="dram", bufs=1, space="DRAM"))
        # PSUM: 8 banks x 2KB/partition, region-packed accumulation chains.
        pkvp = ec(tc.tile_pool(name="pkv", bufs=1, space="PSUM"))  # 1 bank
        pqp = ec(tc.tile_pool(name="pq", bufs=1, space="PSUM"))    # 1 bank
        plp = ec(tc.tile_pool(name="pl", bufs=2, space="PSUM"))    # 2 banks
        spp = ec(tc.tile_pool(name="sp", bufs=1, space="PSUM"))   # 1 bank
        pavp = ec(tc.tile_pool(name="pav", bufs=1, space="PSUM"))  # 1 bank
        pop = ec(tc.tile_pool(name="po", bufs=2, space="PSUM"))    # 2 banks

        # ---- constants ----
        ident = constp.tile([P, P], BF16, name="ident")
        nc.gpsimd.memset(ident, 0.0)
        nc.gpsimd.affine_select(
            out=ident, in_=ident, compare_op=mybir.AluOpType.not_equal,
            fill=1.0, base=0, pattern=[[-1, P]], channel_multiplier=1,
        )
        # masktri[s, c] = NEG iff s > c (0 on the valid s <= c region)
        masktri = constp.tile([P, P], BF16, name="masktri")
        nc.gpsimd.memset(masktri, 0.0)
        nc.gpsimd.affine_select(
            out=masktri, in_=masktri, compare_op=mybir.AluOpType.is_ge,
            fill=NEG, base=0, pattern=[[1, P]], channel_multiplier=-1,
        )
        # maskfull[s, c] (c in [0,256)): NEG where c < 128 or s > c-128
        maskfull = constp.tile([P, BW], BF16, name="maskfull")
        nc.gpsimd.memset(maskfull, 0.0)
        nc.gpsimd.affine_select(
            out=maskfull, in_=maskfull, compare_op=mybir.AluOpType.is_ge,
            fill=NEG, base=-P, pattern=[[1, BW]], channel_multiplier=-1,
        )
        ones_bf = constp.tile([P, 1], BF16, name="ones_bf")
        nc.vector.memset(ones_bf, 1.0)
        ones1 = constp.tile([1, P], BF16, name="ones1")
        nc.vector.memset(ones1, 1.0)
        wrm = constp.tile([P, BW], BF16, name="wrm")
        nc.vector.memset(wrm, 0.0)

        def warm_burst(n):
            pw = pop.tile([P, BW], F32, name="pwb", tag="po")
            for _ in range(n):
                nc.tensor.matmul(out=pw, lhsT=ident, rhs=wrm, start=True, stop=True)

        # PE warmup while first DMAs stream in
        warm_burst(40)

        # ---- batched DMA staging ----
        cos_sb = csp.tile([H2, cfg.T], BF16, name="cos_sb")
        sin_sb = csp.tile([H2, cfg.T], BF16, name="sin_sb")
        wkv_sb = wkvp.tile([P, 2 * KT * cfg.HD], BF16, name="wkv_sb")
        nc.sync.dma_start(wkv_sb, wkv2)
        nc.sync.dma_start(cos_sb, cosT)
        nc.sync.dma_start(sin_sb, sinT)

        xsb = []
        xsb.append(xTp.tile([P, KT * BW], BF16, name="xsb0", tag="xT"))
        nc.sync.dma_start(xsb[0], xb2[0:P, :])

        # wq in 4 chunks of 7 ke-tiles for q-proj(b0) streaming
        wq_sb = wqp.tile([P, KT * HG], BF16, name="wq_sb")
        WQC = KT // 4 * HG  # 6272
        for c in range(4):
            nc.sync.dma_start(wq_sb[:, c * WQC:(c + 1) * WQC],
                              wq2[:, c * WQC:(c + 1) * WQC])

        xsb.append(xTp.tile([P, KT * BW], BF16, name="xsb1", tag="xT"))
        nc.sync.dma_start(xsb[1], xb2[P:2 * P, :])

        wo_sb = wop.tile([P, KT * EO], BF16, name="wo_sb")
        WOC = KT // 2 * EO
        for c in range(2):
            nc.sync.dma_start(wo_sb[:, c * WOC:(c + 1) * WOC],
                              wo2[:, c * WOC:(c + 1) * WOC])

        for b in range(2, NB):
            t = xTp.tile([P, KT * BW], BF16, name=f"xsb{b}", tag="xT")
            nc.sync.dma_start(t, xb2[b * P:(b + 1) * P, :])
            xsb.append(t)

        def xt(ke, b):
            return xsb[b][:, ke * BW:(ke + 1) * BW]

        def wkh(ke):
            return wkv_sb[:, ke * cfg.HD:(ke + 1) * cfg.HD]

        def wvh(ke):
            return wkv_sb[:, KT * cfg.HD + ke * cfg.HD:KT * cfg.HD + (ke + 1) * cfg.HD]

        def wqh(ke, h):
            return wq_sb[:, ke * HG + h * P:ke * HG + (h + 1) * P]

        def woh(kt, e):
            return wo_sb[:, kt * EO + e * P:kt * EO + (e + 1) * P]

        kT = kTp.tile([P, cfg.T], BF16, name="kT")
        vts = [vp.tile([P, cfg.HD], BF16, name=f"v{i}", tag="v")
               for i in range(2 * NB)]

        cc_in = [dramp.tile([P, GQ * BW], BF16, name=f"cc_in{b}")
                 for b in range(NB)]
        cc_out = [dramp.tile([4 * P, GQ * BW], BF16, name=f"cc_out{b}")
                  for b in range(NB)]

        def rope_drain(psum, dst, t0, w, ang0=None):
            """dst[:, t0:t0+w] = rope(psum); psum [128, w] f32, dst bf16.
            ang0: global token offset for the cos/sin tables (default t0)."""
            ang0 = t0 if ang0 is None else ang0
            c = cos_sb[:, ang0:ang0 + w]
            s = sin_sb[:, ang0:ang0 + w]
            p1 = psum[0:H2, :]
            p2 = psum[H2:P, :]
            t1 = rtp.tile([H2, BW], F32, name="t1", tag="rt1")[:, :w]
            t2 = rtp.tile([H2, BW], F32, name="t2", tag="rt2")[:, :w]
            nc.vector.tensor_mul(t1, p1, c)
            nc.vector.tensor_mul(t2, p2, s)
            nc.vector.tensor_sub(dst[0:H2, t0:t0 + w], t1, t2)
            nc.vector.tensor_mul(t1, p2, c)
            nc.vector.tensor_mul(t2, p1, s)
            nc.vector.tensor_add(dst[H2:P, t0:t0 + w], t1, t2)

        qT = {}  # (h, b) -> tile

        def kv_proj(b):
            t0 = b * BW
            # one bank: k in cols [0:256), v sub-tiles in [256:384) / [384:512)
            bank = pkvp.tile([P, 512], F32, name="pkv", tag="pkv")
            psk = bank[:, 0:BW]
            for ke in range(KT):
                nc.tensor.matmul(out=psk, lhsT=wkh(ke), rhs=xt(ke, b),
                                 start=(ke == 0), stop=(ke == KT - 1))
            rope_drain(psk, kT, t0, BW)
            for sub in range(2):
                pv = bank[:, BW + sub * P:BW + (sub + 1) * P]
                for ke in range(KT):
                    nc.tensor.matmul(
                        out=pv, lhsT=xt(ke, b)[:, sub * P:(sub + 1) * P],
                        rhs=wvh(ke),
                        start=(ke == 0), stop=(ke == KT - 1))
                nc.scalar.copy(vts[2 * b + sub], pv)

        def q_proj_heads(b):
            # heads projected into PAIR tiles [128, 512] (head h at cols
            # [(h%2)*256, ...)); last head (6) into a [128, 256] single.
            # PSUM: pairs pack two sequential chains into one bank.
            for pi in range(4):
                heads = [2 * pi, 2 * pi + 1] if pi < 3 else [6]
                W = 256 * len(heads)
                pool = qTp2 if pi < 3 else qTs2
                q = pool.tile([P, W], BF16, name=f"qT{pi}_{b}", tag="qT")
                bank = pqp.tile([P, 512], F32, name=f"pq{pi}", tag="pq")
                for i, h in enumerate(heads):
                    ps = bank[:, i * BW:(i + 1) * BW]
                    for ke in range(KT):
                        nc.tensor.matmul(
                            out=ps, lhsT=wqh(ke, h), rhs=xt(ke, b),
                            start=(ke == 0), stop=(ke == KT - 1))
                    rope_drain(ps, q, i * BW, BW, ang0=b * BW)
                qT[(pi, b)] = q

        def attn_pair(b, pi, qkv_blk):
            """Attention for head pair pi (heads 2pi,2pi+1; pi=3 -> head 6
            alone), token block b. Both heads share this core's kv head, so
            logits/exp/denoms/attnV run on [128, W] with W=512 (256 single).
            All si computed full-width; causality enforced by additive NEG
            masks (exp -> 0), so no column bookkeeping anywhere."""
            W = 512 if pi < 3 else 256
            ng = W // BW
            q = qT[(pi, b)]
            nsi = 2 * (b + 1)
            pts = []
            for si in range(nsi):
                pl = plp.tile([P, 512], F32, name="plb", tag="pl")[:, :W]
                diag = si >= 2 * b
                nc.tensor.matmul(
                    out=pl,
                    lhsT=kT[:, si * P:(si + 1) * P],
                    rhs=q,
                    start=True, stop=not diag,
                )
                if si == 2 * b:
                    for g in range(ng):
                        nc.tensor.matmul(
                            out=pl[:, g * BW:g * BW + P], lhsT=ident,
                            rhs=masktri, start=False, stop=(g == ng - 1))
                elif si == 2 * b + 1:
                    for g in range(ng):
                        nc.tensor.matmul(
                            out=pl[:, g * BW:(g + 1) * BW], lhsT=ident,
                            rhs=maskfull, start=False, stop=(g == ng - 1))
                pt = ptp.tile([P, 512], BF16, name="pt", tag="pt")[:, :W]
                nc.scalar.activation(
                    pt, pl, mybir.ActivationFunctionType.Exp, scale=cfg.scale,
                )
                pts.append(pt)

            sp = spp.tile([1, 512], F32, name="sp", tag="sp")[:, :W]
            for si, pt in enumerate(pts):
                nc.tensor.matmul(
                    out=sp, lhsT=ones_bf, rhs=pt,
                    start=(si == 0), stop=(si == nsi - 1),
                )
            rec_bf = recp.tile([1, 512], BF16, name="rec", tag="rec")[:, :W]
            with nc.allow_low_precision("softmax reciprocal feeds bf16 matmul"):
                nc.vector.reciprocal(out=rec_bf, in_=sp)
            recb_ps = plp.tile([P, 512], F32, name="recbp", tag="pl")[:, :W]
            nc.tensor.matmul(out=recb_ps, lhsT=ones1, rhs=rec_bf,
                             start=True, stop=True)
            # TensorTensor may read only one PSUM operand: drain recb to SBUF
            recb = recp.tile([P, 512], BF16, name="recb", tag="recb")[:, :W]
            nc.vector.tensor_copy(recb, recb_ps)

            pav = pavp.tile([P, 512], F32, name="pav", tag="pav")[:, :W]
            for si, pt in enumerate(pts):
                nc.tensor.matmul(
                    out=pav, lhsT=vts[si], rhs=pt,
                    start=(si == 0), stop=(si == nsi - 1),
                )
            nc.vector.tensor_mul(
                qkv_blk[:, pi * 512:pi * 512 + W], pav, recb)

        def allgather(b):
            nc.gpsimd.collective_compute(
                "AllGather",
                mybir.AluOpType.bypass,
                replica_groups=[[0, 1, 2, 3], [4, 5, 6, 7]],
                ins=[cc_in[b].opt()],
                outs=[cc_out[b].opt()],
            )

        def fetch_qkh(b):
            # gathered qkv^T: one strided DMA [4*128, 7*256] -> [128, 28*256]
            # on the GpSimd queue (waits AG done without blocking Sync DMAs)
            qkh = qkhp.tile([P, NH * BW], BF16, name=f"qkh{b}", tag="qkh")
            nc.gpsimd.dma_start(
                qkh, cc_out[b][:, :].rearrange("(r p) f -> p r f", p=P))
            return qkh

        def oproj(b, qkh):
            osb = osbp.tile([P, cfg.ET * BW], BF16, name=f"osb{b}", tag="osb")
            for e in range(cfg.ET):
                po = pop.tile([P, BW], F32, name="po", tag="po")
                for kt in range(NH):
                    nc.tensor.matmul(
                        out=po, lhsT=woh(kt, e), rhs=qkh[:, kt * BW:(kt + 1) * BW],
                        start=(kt == 0), stop=(kt == NH - 1),
                    )
                nc.scalar.copy(osb[:, e * BW:(e + 1) * BW], po)
            nc.sync.dma_start(
                oT_s[:, b * BW:(b + 1) * BW].rearrange("(e p) c -> p e c", p=P),
                osb)

        def attn_block(b):
            qkv_blk = qkvbp.tile([P, GQ * BW], BF16, name=f"qkvb{b}", tag="qkvb")
            for pi in range(4):
                attn_pair(b, pi, qkv_blk)
            nc.sync.dma_start(cc_in[b], qkv_blk)
            allgather(b)

        # ================= pipeline =================
        qkhs = {}
        for b in range(NB):
            kv_proj(b)
            q_proj_heads(b)
            attn_block(b)
            if b < 2:
                qkhs[b] = fetch_qkh(b)
        for b in range(NB):
            oproj(b, qkhs[b])
            if b + 2 in range(NB):
                qkhs[b + 2] = fetch_qkh(b + 2)


# ======================= host side =======================

_NC_CACHE = {}


def _get_nc(cfg_key=None):
    if cfg_key not in _NC_CACHE:
        _NC_CACHE[cfg_key] = build_kernel(Cfg())
    return _NC_CACHE[cfg_key]


def _rope_tables(segment_ids, cur_ind, T, HD):
    valid = (np.asarray(segment_ids) != 0)
    pos = np.cumsum(valid, axis=-1) - 1 + int(cur_ind)  # [B, T]
    frac = 2.0 * np.arange(HD // 2, dtype=np.float64) / HD
    timescale = THETA ** frac
    ang = pos[..., None].astype(np.float64) / timescale  # [B, T, HD/2]
    cosT = np.transpose(np.cos(ang), (0, 2, 1)).astype(np.float32)  # [B, HD/2, T]
    sinT = np.transpose(np.sin(ang), (0, 2, 1)).astype(np.float32)
    return cosT, sinT


def _pack_km(w, P=128):
    """[K, M] -> [P, (K/P)*M]: tile ke on rows -> partition-major columns."""
    K, M = w.shape
    kt = K // P
    return np.ascontiguousarray(
        w.reshape(kt, P, M).transpose(1, 0, 2).reshape(P, kt * M))


def prepare_in_maps(inputs, cfg=None):
    import ml_dtypes
    bf16 = ml_dtypes.bfloat16
    cfg = cfg or Cfg()
    x = np.asarray(inputs["x"], dtype=np.float32)
    wq = np.asarray(inputs["wq"], dtype=np.float32).astype(bf16)
    wk = np.asarray(inputs["wk"], dtype=np.float32).astype(bf16)
    wv = np.asarray(inputs["wv"], dtype=np.float32).astype(bf16)
    wo = np.asarray(inputs["wo"], dtype=np.float32).astype(bf16)
    seg = np.asarray(inputs["segment_ids"])
    cur = int(np.asarray(inputs["cur_ind"]))

    B, T, EMB = x.shape
    assert (B, T, EMB) == (2, cfg.T, cfg.EMB)
    HG, HD, EO, KT = cfg.HG, cfg.HD, cfg.EO, cfg.KT
    cosT, sinT = _rope_tables(seg, cur, T, HD)
    xT = np.transpose(x, (0, 2, 1)).astype(bf16)  # [B, EMB, T]
    # xb2: [B, NB*P, KT*BW]; block b rows [b*P:(b+1)*P], x tile (ke,b) at
    # cols [ke*BW:(ke+1)*BW]
    xb2 = np.ascontiguousarray(
        xT.reshape(B, KT, P, NB, BW).transpose(0, 3, 2, 1, 4).reshape(
            B, NB * P, KT * BW))

    in_maps = []
    for c in range(8):
        b, j = c // 4, c % 4
        wkj = np.ascontiguousarray(wk[:, j * HD:(j + 1) * HD])
        wvj = np.ascontiguousarray(wv[:, j * HD:(j + 1) * HD])
        wkv2 = np.concatenate([_pack_km(wkj), _pack_km(wvj)], axis=1)
        in_maps.append({
            "xb2": xb2[b],
            "wq2": _pack_km(np.ascontiguousarray(wq[:, j * HG:(j + 1) * HG])),
            "wkv2": np.ascontiguousarray(wkv2),
            "wo2": _pack_km(np.ascontiguousarray(wo[:, j * EO:(j + 1) * EO])),
            "cosT": np.ascontiguousarray(cosT[b]).astype(bf16),
            "sinT": np.ascontiguousarray(sinT[b]).astype(bf16),
        })
    return in_maps


def assemble_out(results, cfg=None):
    cfg = cfg or Cfg()
    out = np.empty((2, cfg.T, cfg.EMB), np.float32)
    for c in range(8):
        b, j = c // 4, c % 4
        out[b, :, j * cfg.EO:(j + 1) * cfg.EO] = results[c]["oT_s"].T.astype(np.float32)
    return out


def kernel(**inputs):
    cfg = Cfg()
    in_maps = prepare_in_maps(inputs, cfg)
    nc = _get_nc()
    res = run_bass_kernel_spmd(nc, in_maps, core_ids=list(range(8)))
    return assemble_out(res.results, cfg)
